# revision 9
# baseline (speedup 1.0000x reference)
"""Trainium2 Bass kernel for a pre-LN transformer block (B=2,T=2048,C=768,H=12,F=3072).

Sharding: pure data-parallel over 8 cores = 2 batches x 4 query-groups.
Core (b, g) handles query tiles {g, g+4, g+8, g+12} (128 rows each), strided so
every core runs an identical SPMD program; causality differences are carried by
per-core mask *data*. k/v are recomputed per core from the full batch (no
cross-core communication).

Layouts: activations row-major for LN/residuals, feature-major (via PE
transpose) for matmul contraction. Attention uses an S^T (key-major) sweep: no
softmax max-subtraction (|S| < 1 here), denominator via a ones-column appended
to v, normalization deferred to the [64,512] per-head output. Matmuls run
fp32r (full-rate fp32); q/k/v/E in bf16.
"""
import sys

sys.path.insert(0, "/opt/trn_rl_repo")
sys.path.insert(0, "/opt/trn_rl_repo/concourse")

from contextlib import ExitStack

import numpy as np

import concourse.bass as bass
import concourse.tile as tile
from concourse import bacc, mybir
from concourse.bass_utils import run_bass_kernel_spmd
from concourse.masks import make_identity

B, T, C, H, D, F = 2, 2048, 768, 12, 64, 3072
EPS = 1e-5
NCORES = 8
QUAD = 4          # cores per batch
NJ = 4            # q-tiles of 128 per core
R = 512           # rows per core
NRT = T // 128    # 16 row tiles of x_full
NCB = C // 128    # 6 feature chunks
NFT = F // 128    # 24 mlp feature chunks

F32 = mybir.dt.float32
F32R = mybir.dt.float32r
BF16 = mybir.dt.bfloat16


def _r(ap):
    return ap.bitcast(F32R)


def build_program():
    nc = bacc.Bacc("TRN2", target_bir_lowering=False, debug=False,
                   num_devices=NCORES)
    # ---- DRAM I/O ----
    x_full = nc.dram_tensor("x_full", (T, C), F32, kind="ExternalInput").ap()
    x_own = nc.dram_tensor("x_own", (R, C), F32, kind="ExternalInput").ap()
    msk_d = nc.dram_tensor("msk", (128, 512), BF16, kind="ExternalInput").ap()
    wq_d = nc.dram_tensor("wq", (C, C), F32R, kind="ExternalInput").ap()
    wk_d = nc.dram_tensor("wk", (C, C), F32R, kind="ExternalInput").ap()
    wv_d = nc.dram_tensor("wv", (C, C), F32R, kind="ExternalInput").ap()
    wp_d = nc.dram_tensor("wp", (C, C), F32R, kind="ExternalInput").ap()
    cqk_d = nc.dram_tensor("cqk", (128, 12), F32, kind="ExternalInput").ap()
    cv_d = nc.dram_tensor("cv", (1, C), F32R, kind="ExternalInput").ap()
    bp_d = nc.dram_tensor("bp", (1, C), F32R, kind="ExternalInput").ap()
    w1_d = nc.dram_tensor("w1", (C, F), F32R, kind="ExternalInput").ap()
    c1_d = nc.dram_tensor("c1", (128, NFT), F32, kind="ExternalInput").ap()
    w2_d = nc.dram_tensor("w2", (F, C), F32R, kind="ExternalInput").ap()
    b2c_d = nc.dram_tensor("b2c", (128, NCB), F32, kind="ExternalInput").ap()
    ones_d = nc.dram_tensor("ones1", (1, 512), F32R, kind="ExternalInput").ap()
    out_d = nc.dram_tensor("out", (R, C), F32, kind="ExternalOutput").ap()

    Exp = mybir.ActivationFunctionType.Exp
    Relu = mybir.ActivationFunctionType.Relu
    Copy = mybir.ActivationFunctionType.Copy
    Ident = mybir.ActivationFunctionType.Identity
    Sqrt = mybir.ActivationFunctionType.Sqrt
    MUL = mybir.AluOpType.mult
    ADD = mybir.AluOpType.add
    SUB = mybir.AluOpType.subtract

    with tile.TileContext(nc) as tc, ExitStack() as top:
        const = top.enter_context(tc.tile_pool(name="const", bufs=1))
        ident = const.tile([128, 128], F32)
        make_identity(nc, ident[:])
        epsc = const.tile([128, 1], F32)
        nc.vector.memset(epsc[:], EPS)
        ones = const.tile([1, 512], F32R)
        nc.sync.dma_start(ones[:], ones_d)
        msk = const.tile([128, 512], BF16)
        nc.sync.dma_start(msk[:], msk_d)
        cqk = const.tile([128, 12], F32)
        nc.sync.dma_start(cqk[:], cqk_d)
        cv = const.tile([1, C], F32R)
        nc.sync.dma_start(cv[:], cv_d)
        bp = const.tile([1, C], F32R)
        nc.sync.dma_start(bp[:], bp_d)
        c1 = const.tile([128, NFT], F32)
        nc.sync.dma_start(c1[:], c1_d)
        b2c = const.tile([128, NCB], F32)
        nc.sync.dma_start(b2c[:], b2c_d)

        # persistent activation tensors
        act = top.enter_context(tc.tile_pool(name="act", bufs=1))
        xo_sb = act.tile([128, NJ * C], F32)        # x_own rm, 12KB/part
        qfm = act.tile([128, NCB * R], BF16)        # q feat-major, 6KB
        kfm = act.tile([128, NCB * T], BF16)        # k feat-major, 24KB
        vrm = act.tile([128, NRT * H * 65], BF16)   # v' row-major 65-blocks, 24.4KB
        x2 = act.tile([128, NJ * C], F32)           # x + sa, 12KB

        stats = top.enter_context(tc.tile_pool(name="stats", bufs=4))

        def ln_tile(x_ap):
            st = stats.tile([128, 12], F32, tag="lnst")
            nc.vector.bn_stats(st[:, 0:6], x_ap[:, 0:384])
            nc.vector.bn_stats(st[:, 6:12], x_ap[:, 384:768])
            mv = stats.tile([128, 2], F32, tag="lnmv")
            nc.vector.bn_aggr(mv[:], st[:].rearrange("p (g k) -> p g k", g=2))
            sd = stats.tile([128, 1], F32, tag="lnsd")
            nc.scalar.activation(sd[:], mv[:, 1:2], Sqrt, bias=epsc[:])
            rr = stats.tile([128, 1], F32, tag="lnrr")
            nc.vector.reciprocal(rr[:], sd[:])
            zt = stats.tile([128, C], F32, tag="lnz")
            nc.vector.tensor_scalar(zt[:], x_ap, mv[:, 0:1], rr[:],
                                    op0=SUB, op1=MUL)
            return zt

        with ExitStack() as phase1:
            zpool = phase1.enter_context(tc.tile_pool(name="zfm", bufs=1))
            zfm = zpool.tile([128, NCB * T], F32R)       # z feat-major, 48KB
            zofm = zpool.tile([128, NCB * R], F32R)      # z_own feat-major, 12KB

            ld = phase1.enter_context(tc.tile_pool(name="ld", bufs=3))
            wst = phase1.enter_context(tc.tile_pool(name="wst", bufs=12))
            wvh = phase1.enter_context(tc.tile_pool(name="wvh", bufs=6))
            tp = phase1.enter_context(tc.tile_pool(name="tp", bufs=2, space="PSUM"))
            kqp = phase1.enter_context(tc.tile_pool(name="kqp", bufs=2, space="PSUM"))
            vp_ = phase1.enter_context(tc.tile_pool(name="vp", bufs=2, space="PSUM"))

            # ---- Stage A: LN1 + transpose (x_full -> zfm, x_own -> zofm) ----
            for rt in range(NRT):
                xt = ld.tile([128, C], F32, tag="xf")
                nc.sync.dma_start(xt[:], x_full[128 * rt:128 * rt + 128, :])
                zt = ln_tile(xt[:])
                for cb in range(NCB):
                    pt = tp.tile([128, 128], F32, tag="zt")
                    nc.tensor.transpose(pt[:], zt[:, 128 * cb:128 * cb + 128], ident[:])
                    nc.any.tensor_copy(zfm[:, T * cb + 128 * rt: T * cb + 128 * rt + 128], pt[:])
            for j in range(NJ):
                nc.sync.dma_start(xo_sb[:, C * j:C * (j + 1)],
                                  x_own[128 * j:128 * j + 128, :])
                zt = ln_tile(xo_sb[:, C * j:C * (j + 1)])
                for cb in range(NCB):
                    pt = tp.tile([128, 128], F32, tag="zt")
                    nc.tensor.transpose(pt[:], zt[:, 128 * cb:128 * cb + 128], ident[:])
                    nc.any.tensor_copy(zofm[:, R * cb + 128 * j: R * cb + 128 * j + 128], pt[:])

            # ---- Stage B: k (fm), v (rm + ones cols), q (fm) ----
            for ct in range(NCB):          # k column tiles (2 heads each)
                wk_t = []
                for cb in range(NCB):
                    w = wst.tile([128, 128], F32R, tag="wk")
                    nc.sync.dma_start(w[:], wk_d[128 * cb:128 * cb + 128,
                                                 128 * ct:128 * ct + 128])
                    wk_t.append(w)
                for rc in range(T // 512):
                    kp = kqp.tile([128, 512], F32, tag="kp")
                    for cb in range(NCB):
                        nc.tensor.matmul(kp[:], (wk_t[cb][:]),
                                         (zfm[:, T * cb + 512 * rc: T * cb + 512 * rc + 512]),
                                         start=(cb == 0), stop=(cb == NCB - 1))
                    nc.scalar.activation(kfm[:, T * ct + 512 * rc: T * ct + 512 * rc + 512],
                                         kp[:], Ident, bias=cqk[:, 6 + ct:7 + ct])
            # ones columns of v' (col 64 of each 65-block)
            nc.vector.memset(vrm[:].rearrange("p (n k) -> p n k", k=65)[:, :, 64:65], 1.0)
            wv_t = []
            for cb in range(NCB):
                w = wvh.tile([128, C], F32R, tag="wv")
                nc.sync.dma_start(w[:], wv_d[128 * cb:128 * cb + 128, :])
                wv_t.append(w)
            for rt in range(NRT):          # v row tiles
                for hf in range(2):        # halves: heads 0-5 / 6-11
                    vp = vp_.tile([128, 384], F32, tag="vp")
                    for cb in range(NCB):
                        nc.tensor.matmul(vp[:],
                                         (zfm[:, T * cb + 128 * rt: T * cb + 128 * rt + 128]),
                                         (wv_t[cb][:, 384 * hf:384 * hf + 384]),
                                         start=(cb == 0), stop=False,
                                         skip_group_check=True)
                    nc.tensor.matmul(vp[:], (ones[0:1, 0:128]),
                                     (cv[0:1, 384 * hf:384 * hf + 384]),
                                     start=False, stop=True, skip_group_check=True)
                    dst = vrm[:, 65 * (H * rt + 6 * hf): 65 * (H * rt + 6 * hf) + 65 * 6]
                    nc.any.tensor_copy(
                        dst.rearrange("p (h k) -> p h k", k=65)[:, :, 0:64],
                        vp[:].rearrange("p (h k) -> p h k", k=64))
            for ct in range(NCB):          # q column tiles
                qp = kqp.tile([128, 512], F32, tag="qp")
                for cb in range(NCB):
                    w = wst.tile([128, 128], F32R, tag="wq")
                    nc.sync.dma_start(w[:], wq_d[128 * cb:128 * cb + 128,
                                                 128 * ct:128 * ct + 128])
                    nc.tensor.matmul(qp[:], (w[:]),
                                     (zofm[:, R * cb: R * cb + R]),
                                     start=(cb == 0), stop=(cb == NCB - 1))
                nc.scalar.activation(qfm[:, R * ct: R * ct + R], qp[:], Ident,
                                     bias=cqk[:, ct:ct + 1])

        # ---- Stage C: attention (S^T sweep) ----
        late = tc.alloc_tile_pool(name="late", bufs=1)
        z2fm = late.tile([128, NCB * R], F32R)           # 12KB
        out_sb = late.tile([128, NJ * C], F32)          # 12KB
        afm_pool = tc.alloc_tile_pool(name="afm", bufs=1)
        afm = afm_pool.tile([128, NCB * R], F32R)        # attn out fm, 12KB
        with ExitStack() as phase2:
            ep = phase2.enter_context(tc.tile_pool(name="ep", bufs=3))
            sp_ = phase2.enter_context(tc.tile_pool(name="sp", bufs=3, space="PSUM"))
            app = phase2.enter_context(tc.tile_pool(name="app", bufs=2, space="PSUM"))
            bcp = phase2.enter_context(tc.tile_pool(name="bcp", bufs=2, space="PSUM"))
            for h in range(H):
                hb, ho = h // 2, 64 * (h % 2)
                ap = app.tile([128, 512], F32, tag="ap")
                for c in range(4):
                    n = 512 - 128 * c
                    for kb in range(4):
                        ko = T * hb + 512 * c + 128 * kb
                        sp = sp_.tile([128, 512], F32, tag="sp")
                        nc.tensor.matmul(sp[:, 0:n],
                                         kfm[ho:ho + 64, ko:ko + 128],
                                         qfm[ho:ho + 64, R * hb + 128 * c: R * hb + 512],
                                         start=True, stop=True)
                        e = ep.tile([128, 512], BF16, tag="e")
                        nc.scalar.activation(e[:, 0:n], sp[:, 0:n], Exp)
                        nc.vector.tensor_tensor(e[:, 0:128], e[:, 0:128],
                                                msk[:, 128 * kb:128 * kb + 128], op=MUL)
                        vo = 65 * (H * (4 * c + kb) + h)
                        nc.tensor.matmul(ap[0:65, 128 * c:512],
                                         vrm[:, vo:vo + 65],
                                         e[:, 0:n],
                                         start=(c == 0 and kb == 0),
                                         stop=(c == 3 and kb == 3),
                                         skip_group_check=True)
                invd = ep.tile([1, 512], F32R, tag="invd")
                with nc.allow_low_precision(reason="fp32r invd for broadcast matmul"):
                    nc.vector.reciprocal(invd[:], ap[64:65, :])
                bc = bcp.tile([128, 512], F32, tag="bc")
                nc.tensor.matmul(bc[0:64, :], (ones[0:1, 0:64]), (invd[:]),
                                 start=True, stop=True)
                raw = ep.tile([64, 512], F32, tag="raw")
                nc.any.tensor_copy(raw[:], ap[0:64, :])
                nc.vector.tensor_tensor(afm[ho:ho + 64, R * hb: R * hb + R],
                                        raw[:], bc[0:64, :], op=MUL)

        # ---- Stage D: proj + residual; Stage E: LN2 ----
        with ExitStack() as phase3:
            wph = phase3.enter_context(tc.tile_pool(name="wph", bufs=6))
            pp_ = phase3.enter_context(tc.tile_pool(name="pp", bufs=2, space="PSUM"))
            tp2 = phase3.enter_context(tc.tile_pool(name="tp2", bufs=2, space="PSUM"))
            wp_t = []
            for cb in range(NCB):
                w = wph.tile([128, C], F32R, tag="wp")
                nc.sync.dma_start(w[:], wp_d[128 * cb:128 * cb + 128, :])
                wp_t.append(w)
            for j in range(NJ):
                pp = pp_.tile([128, C], F32, tag="pp")
                for no, nn in ((0, 512), (512, 256)):
                    for cb in range(NCB):
                        nc.tensor.matmul(pp[:, no:no + nn],
                                         (afm[:, R * cb + 128 * j: R * cb + 128 * j + 128]),
                                         (wp_t[cb][:, no:no + nn]),
                                         start=(cb == 0), stop=False,
                                         skip_group_check=True)
                    nc.tensor.matmul(pp[:, no:no + nn], (ones[0:1, 0:128]),
                                     (bp[0:1, no:no + nn]), start=False, stop=True,
                                     skip_group_check=True)
                nc.vector.tensor_tensor(x2[:, C * j:C * (j + 1)],
                                        xo_sb[:, C * j:C * (j + 1)], pp[:], op=ADD)
                zt = ln_tile(x2[:, C * j:C * (j + 1)])
                for cb in range(NCB):
                    pt = tp2.tile([128, 128], F32, tag="zt2")
                    nc.tensor.transpose(pt[:], zt[:, 128 * cb:128 * cb + 128], ident[:])
                    nc.any.tensor_copy(z2fm[:, R * cb + 128 * j: R * cb + 128 * j + 128], pt[:])
        afm_pool.release()

        # ---- Stage F: MLP1; Stage G: MLP2 + relu + transpose + residual ----
        with ExitStack() as phase4:
            a1pool = phase4.enter_context(tc.tile_pool(name="a1", bufs=1))
            a1 = a1pool.tile([128, NFT * R], F32R)       # 48KB
            w1st = phase4.enter_context(tc.tile_pool(name="w1st", bufs=8))
            w2st = phase4.enter_context(tc.tile_pool(name="w2st", bufs=8))
            mp_ = phase4.enter_context(tc.tile_pool(name="mp", bufs=2, space="PSUM"))
            fp_ = phase4.enter_context(tc.tile_pool(name="fp", bufs=2, space="PSUM"))
            ftp = phase4.enter_context(tc.tile_pool(name="ftp", bufs=2, space="PSUM"))
            ffs_ = phase4.enter_context(tc.tile_pool(name="ffs", bufs=2))
            for ft in range(NFT):
                mp = mp_.tile([128, R], F32, tag="mp")
                for cb in range(NCB):
                    w = w1st.tile([128, 128], F32R, tag="w1")
                    nc.sync.dma_start(w[:], w1_d[128 * cb:128 * cb + 128,
                                                 128 * ft:128 * ft + 128])
                    nc.tensor.matmul(mp[:], (w[:]), (z2fm[:, R * cb: R * cb + R]),
                                     start=(cb == 0), stop=(cb == NCB - 1))
                nc.scalar.activation(a1[:, R * ft: R * ft + R], mp[:], Relu,
                                     bias=c1[:, ft:ft + 1])
            for ct in range(NCB):
                fp = fp_.tile([128, R], F32, tag="fp")
                for ft in range(NFT):
                    w = w2st.tile([128, 128], F32R, tag="w2")
                    nc.sync.dma_start(w[:], w2_d[128 * ft:128 * ft + 128,
                                                 128 * ct:128 * ct + 128])
                    nc.tensor.matmul(fp[:], (w[:]), (a1[:, R * ft: R * ft + R]),
                                     start=(ft == 0), stop=(ft == NFT - 1))
                ffs = ffs_.tile([128, R], F32, tag="ffs")
                nc.scalar.activation(ffs[:], fp[:], Relu, bias=b2c[:, ct:ct + 1])
                for j in range(NJ):
                    pt = ftp.tile([128, 128], F32, tag="ftp")
                    nc.tensor.transpose(pt[:], ffs[:, 128 * j:128 * j + 128], ident[:])
                    nc.vector.tensor_tensor(
                        out_sb[:, C * j + 128 * ct: C * j + 128 * ct + 128],
                        x2[:, C * j + 128 * ct: C * j + 128 * ct + 128],
                        pt[:], op=ADD)

        for j in range(NJ):
            nc.sync.dma_start(out_d[128 * j:128 * j + 128, :],
                              out_sb[:, C * j:C * (j + 1)])
        late.release()

    nc.finalize()
    return nc


_CACHE = {}


def _get_nc():
    if "nc" not in _CACHE:
        _CACHE["nc"] = build_program()
    return _CACHE["nc"]


def _host_prep(inputs):
    import ml_dtypes
    x = np.ascontiguousarray(np.asarray(inputs["x"], np.float32))
    Wq = np.asarray(inputs["Wq"], np.float32).transpose(1, 0, 2).reshape(C, C)
    Wk = np.asarray(inputs["Wk"], np.float32).transpose(1, 0, 2).reshape(C, C)
    Wv = np.asarray(inputs["Wv"], np.float32).transpose(1, 0, 2).reshape(C, C)
    g1 = np.asarray(inputs["ln1_g"], np.float32)
    b1l = np.asarray(inputs["ln1_b"], np.float32)
    g2 = np.asarray(inputs["ln2_g"], np.float32)
    b2l = np.asarray(inputs["ln2_b"], np.float32)
    s = np.float32(C ** -0.5)
    wq = np.ascontiguousarray(g1[:, None] * Wq * s)
    wk = np.ascontiguousarray(g1[:, None] * Wk)
    wv = np.ascontiguousarray(g1[:, None] * Wv)
    cq = (b1l @ Wq) * s
    ck = b1l @ Wk
    cv = np.ascontiguousarray((b1l @ Wv).reshape(1, C))
    cqk = np.ascontiguousarray(
        np.concatenate([cq.reshape(NCB, 128).T, ck.reshape(NCB, 128).T], axis=1))
    W1 = np.asarray(inputs["W1"], np.float32)
    w1 = np.ascontiguousarray(g2[:, None] * W1)
    c1 = np.ascontiguousarray((b2l @ W1 + np.asarray(inputs["b1"], np.float32))
                              .reshape(NFT, 128).T)
    wp = np.ascontiguousarray(np.asarray(inputs["Wp"], np.float32))
    bp = np.ascontiguousarray(np.asarray(inputs["bp"], np.float32).reshape(1, C))
    w2 = np.ascontiguousarray(np.asarray(inputs["W2"], np.float32))
    b2c = np.ascontiguousarray(
        np.asarray(inputs["b2"], np.float32).reshape(NCB, 128).T)

    in_maps = []
    row_idx = []
    for core in range(NCORES):
        b, g = core // QUAD, core % QUAD
        rows = np.concatenate([np.arange(128 * (g + 4 * j), 128 * (g + 4 * j) + 128)
                               for j in range(NJ)])
        row_idx.append((b, rows))
        kl = np.arange(128)[:, None]
        ql = np.arange(128)[None, :]
        msk = np.zeros((128, 512), np.float32)
        for kb in range(4):
            msk[:, 128 * kb:128 * kb + 128] = (kl <= 128 * (g - kb) + ql)
        in_maps.append({
            "x_full": x[b],
            "x_own": np.ascontiguousarray(x[b][rows]),
            "msk": msk.astype(ml_dtypes.bfloat16),
            "wq": wq, "wk": wk, "wv": wv, "wp": wp,
            "cqk": cqk, "cv": cv, "bp": bp,
            "w1": w1, "c1": c1, "w2": w2, "b2c": b2c,
            "ones1": np.ones((1, 512), np.float32),
        })
    return in_maps, row_idx


def _run(inputs, trace=False):
    nc = _get_nc()
    in_maps, row_idx = _host_prep(inputs)
    res = run_bass_kernel_spmd(nc, in_maps, core_ids=list(range(NCORES)),
                               trace=trace)
    out = np.zeros((B, T, C), np.float32)
    for core in range(NCORES):
        b, rows = row_idx[core]
        out[b][rows] = res.results[core]["out"]
    return out, res


def kernel(**inputs):
    out, _ = _run(inputs, trace=False)
    return out
